# revision 19
# baseline (speedup 1.0000x reference)
"""Trainium2 Bass kernel for ConditionalHierarchicalCrossEntropyLoss.

Data-parallel: shard y_pred/y_true along batch across 8 NeuronCores;
replicate the tiny per-class table; sum the per-core partials on host.

Per 128-row block on each core:
  1. DMA y_true block [128, 8192] -> SBUF. Two-level argmax: DVE
     per-chunk max reduce [128, 64x128] -> [128, 64]; InstMax+InstMaxIndex
     pick the winning 128-wide chunk; indirect-DMA re-gathers that 512B
     chunk from HBM; InstMaxIndex inside it gives the label
     (first-occurrence at every level, matching jnp.argmax).
  2. gpsimd indirect DMA gathers the per-class record
     [path0..5, len, -, wm0..4] from a host-built DRAM table, where
     wm_k = class_w * exp(-0.1*(len-1-k)) * [k < len-1].
  3. gpsimd indirect DMA gathers y_pred[row, path_k] (6 scalars/row)
     from HBM using flat offsets row*8192 + path_k (one offset per
     partition per gather -- the HW DGE constraint).
  4. DMA y_pred block -> SBUF; ACT exp-accumulate gives the softmax
     denominator Z per row (no max-subtraction: inputs ~ randn).
  5. Tiny [128, <=8] ops: suffix sums, conditional probs, ln, weighted
     row loss, accumulated across blocks.
Output per core: [128, 1] partials; host: loss = -sum(partials)/B.
"""

import numpy as np

import concourse.bacc as bacc
import concourse.bass as bass
import concourse.tile as tile
from concourse import mybir

N_CORES = 8
B = 8192          # batch
C = 8192          # classes
RPC = B // N_CORES  # rows per core
P = 128           # partitions / rows per block
NBLK = RPC // P   # blocks per core
D = 6             # max tree depth (padded path length)
NCHUNK = 64       # chunks per row for two-level argmax
CW = C // NCHUNK  # chunk width (128)
EPS = 1e-8
DEPTH_PARAM = 0.1

f32 = mybir.dt.float32
u32 = mybir.dt.uint32

Alu = mybir.AluOpType
Act = mybir.ActivationFunctionType


def _body(tc, yp_d, yt_d, tab_d, cst_d, out_d, dbg=None, repeats=1):
    nc = tc.nc
    with (
        tc.tile_pool(name="big", bufs=2) as big,
        tc.tile_pool(name="small", bufs=3) as small,
        tc.tile_pool(name="single", bufs=1) as single,
    ):
        cst = single.tile([P, 16], f32)
        nc.sync.dma_start(out=cst[:], in_=cst_d)
        kvec = cst[:, 0:6]   # 0..5 per row

        acc = single.tile([P, 1], f32)

        for it in range(repeats * NBLK):
            rep, b = divmod(it, NBLK)
            if b == 0:
                nc.vector.memset(acc[:], 0.0)
            rows = slice(b * P, (b + 1) * P)
            rowbase = cst[:, 8 + b:9 + b]   # (b*128+p)*8192 per partition

            # ---- two-level argmax of y_true[rows] ----
            yt = big.tile([P, C], f32, tag="yt")
            nc.sync.dma_start(out=yt[:], in_=yt_d[rows, :])
            cmax = small.tile([P, NCHUNK], f32)
            nc.vector.tensor_reduce(
                out=cmax[:],
                in_=yt[:].rearrange("p (a b) -> p a b", b=CW),
                axis=mybir.AxisListType.X, op=Alu.max,
            )
            m8 = small.tile([P, 8], f32)
            nc.vector.max(m8[:], cmax[:])
            cidx = small.tile([P, 8], u32)
            nc.vector.max_index(cidx[:], m8[:], cmax[:])

            cidx_f = small.tile([P, 1], f32)
            nc.vector.tensor_copy(out=cidx_f[:], in_=cidx[:, 0:1])
            offc_f = small.tile([P, 1], f32)
            nc.scalar.activation(out=offc_f[:], in_=cidx_f[:],
                                 func=Act.Identity, scale=float(CW),
                                 bias=rowbase)
            offc_u = small.tile([P, 1], u32)
            nc.vector.tensor_copy(out=offc_u[:], in_=offc_f[:])
            chunk = small.tile([P, CW], f32)
            nc.gpsimd.indirect_dma_start(
                out=chunk[:], out_offset=None, in_=yt_d,
                in_offset=bass.IndirectOffsetOnAxis(ap=offc_u[:], axis=1),
            )
            inner = small.tile([P, 8], u32)
            nc.vector.max_index(inner[:], m8[:], chunk[:])
            inner_f = small.tile([P, 1], f32)
            nc.vector.tensor_copy(out=inner_f[:], in_=inner[:, 0:1])
            lab_f = small.tile([P, 1], f32)
            nc.vector.scalar_tensor_tensor(
                out=lab_f[:], in0=cidx_f[:], scalar=float(CW), in1=inner_f[:],
                op0=Alu.mult, op1=Alu.add,
            )
            lab_u = small.tile([P, 1], u32)
            nc.vector.tensor_copy(out=lab_u[:], in_=lab_f[:])

            # ---- per-class record gather ----
            rec = small.tile([P, 16], f32)
            nc.gpsimd.indirect_dma_start(
                out=rec[:], out_offset=None, in_=tab_d,
                in_offset=bass.IndirectOffsetOnAxis(ap=lab_u[:, 0:1], axis=0),
            )

            # ---- gather y_pred[row, path_k] from HBM ----
            offf = small.tile([P, D], f32)
            nc.scalar.activation(out=offf[:], in_=rec[:, 0:6],
                                 func=Act.Identity, bias=rowbase)
            offu = small.tile([P, D], u32)
            nc.vector.tensor_copy(out=offu[:], in_=offf[:])
            g = small.tile([P, D], f32)
            for k in range(D):
                nc.gpsimd.indirect_dma_start(
                    out=g[:, k:k + 1], out_offset=None, in_=yp_d,
                    in_offset=bass.IndirectOffsetOnAxis(
                        ap=offu[:, k:k + 1], axis=1),
                )

            # ---- softmax denominator Z ----
            yp = big.tile([P, C], f32, tag="yp")
            nc.sync.dma_start(out=yp[:], in_=yp_d[rows, :])
            z = small.tile([P, 1], f32)
            nc.scalar.activation(out=yp[:], in_=yp[:], func=Act.Exp,
                                 accum_out=z[:])
            rz = small.tile([P, 1], f32)
            nc.vector.reciprocal(rz[:], z[:])

            # ---- probs_k = exp(g_k)/Z masked to k < len ----
            eg = small.tile([P, D], f32)
            nc.scalar.activation(out=eg[:], in_=g[:], func=Act.Exp)
            nm = small.tile([P, D], f32)
            nc.vector.tensor_scalar(
                out=nm[:], in0=kvec, scalar1=rec[:, 6:7], scalar2=None,
                op0=Alu.is_lt,
            )
            probs = small.tile([P, D], f32)
            nc.vector.scalar_tensor_tensor(
                out=probs[:], in0=eg[:], scalar=rz[:, 0:1], in1=nm[:],
                op0=Alu.mult, op1=Alu.mult,
            )

            # ---- suffix sums s_k = sum_{j>=k} probs_j (in place) ----
            for k in range(D - 2, -1, -1):
                nc.scalar.add(probs[:, k:k + 1], probs[:, k:k + 1],
                              probs[:, k + 1:k + 2])

            # ---- cond_k = s_k/(s_{k+1}+EPS); row loss via host LUT ----
            sn = small.tile([P, D - 1], f32)
            nc.scalar.activation(out=sn[:], in_=probs[:, 1:6],
                                 func=Act.Identity, bias=cst[:, 7:8])
            rsn = small.tile([P, D - 1], f32)
            nc.vector.reciprocal(rsn[:], sn[:])
            cond = small.tile([P, D - 1], f32)
            nc.vector.tensor_tensor(out=cond[:], in0=probs[:, 0:5],
                                    in1=rsn[:], op=Alu.mult)
            lc = small.tile([P, D - 1], f32)
            nc.scalar.activation(out=lc[:], in_=cond[:], func=Act.Ln,
                                 bias=cst[:, 7:8])
            t2 = small.tile([P, D - 1], f32)
            pl = small.tile([P, 1], f32)
            nc.vector.scalar_tensor_tensor(
                out=t2[:], in0=lc[:], scalar=1.0, in1=rec[:, 8:13],
                op0=Alu.mult, op1=Alu.mult, accum_out=pl[:],
            )
            nc.vector.tensor_tensor(out=acc[:], in0=acc[:], in1=pl[:],
                                    op=Alu.add)

            if dbg is not None:
                lab_d, z_d, g_d, pl_d, off_d = dbg
                nc.sync.dma_start(out=lab_d[rows, :], in_=lab_f[:])
                nc.sync.dma_start(out=z_d[rows, :], in_=z[:])
                nc.sync.dma_start(out=g_d[rows, :], in_=g[:])
                nc.sync.dma_start(out=pl_d[rows, :], in_=pl[:])
                nc.sync.dma_start(out=off_d[rows, :], in_=offu[:])

        nc.sync.dma_start(out=out_d, in_=acc[:])


def build_bass(debug_outs=False, repeats=1):
    nc = bacc.Bacc("TRN2", target_bir_lowering=False, debug=False,
                   enable_asserts=False)
    yp = nc.dram_tensor("y_pred_s", [RPC, C], f32, kind="ExternalInput")
    yt = nc.dram_tensor("y_true_s", [RPC, C], f32, kind="ExternalInput")
    tab = nc.dram_tensor("table", [C, 16], f32, kind="ExternalInput")
    cst = nc.dram_tensor("consts", [P, 16], f32, kind="ExternalInput")
    out = nc.dram_tensor("partial", [P, 1], f32, kind="ExternalOutput")
    dbg = None
    if debug_outs:
        dbg = (
            nc.dram_tensor("lab_dbg", [RPC, 1], f32, kind="ExternalOutput").ap(),
            nc.dram_tensor("z_dbg", [RPC, 1], f32, kind="ExternalOutput").ap(),
            nc.dram_tensor("g_dbg", [RPC, D], f32, kind="ExternalOutput").ap(),
            nc.dram_tensor("pl_dbg", [RPC, 1], f32, kind="ExternalOutput").ap(),
            nc.dram_tensor("off_dbg", [RPC, D], u32, kind="ExternalOutput").ap(),
        )
    with tile.TileContext(nc) as tc:
        _body(tc, yp.ap(), yt.ap(), tab.ap(), cst.ap(), out.ap(), dbg,
              repeats=repeats)
    nc.compile()
    return nc


def make_host_tables(class_w, tree_paths, tree_lens):
    class_w = np.asarray(class_w, np.float64)
    lens = np.asarray(tree_lens, np.float64)
    table = np.zeros((C, 16), np.float32)
    table[:, 0:6] = np.asarray(tree_paths, np.float32)
    table[:, 6] = lens.astype(np.float32)
    k = np.arange(D - 1, dtype=np.float64)
    h = lens[:, None] - 1.0 - k[None, :]
    w = np.exp(-DEPTH_PARAM * h.astype(np.float32).astype(np.float64))
    valid = k[None, :] < (lens[:, None] - 1.0)
    table[:, 8:13] = (class_w[:, None] * w * valid).astype(np.float32)

    consts = np.zeros((P, 16), np.float32)
    consts[:, 0:6] = np.arange(D, dtype=np.float32)[None, :]
    consts[:, 6] = 1.0
    consts[:, 7] = EPS
    p_idx = np.arange(P, dtype=np.float32)
    for b in range(NBLK):
        consts[:, 8 + b] = (b * P + p_idx) * C
    return table, consts


def make_in_maps(y_pred, y_true, table, consts):
    y_pred = np.ascontiguousarray(np.asarray(y_pred, np.float32))
    y_true = np.ascontiguousarray(np.asarray(y_true, np.float32))
    in_maps = []
    for c in range(N_CORES):
        in_maps.append({
            "y_pred_s": y_pred[c * RPC:(c + 1) * RPC],
            "y_true_s": y_true[c * RPC:(c + 1) * RPC],
            "table": table,
            "consts": consts,
        })
    return in_maps


_NC = None


def kernel(y_pred, y_true, class_w, tree_paths, tree_lens):
    global _NC
    from concourse.bass_utils import run_bass_kernel_spmd
    if _NC is None:
        _NC = build_bass()
    table, consts = make_host_tables(class_w, tree_paths, tree_lens)
    in_maps = make_in_maps(y_pred, y_true, table, consts)
    res = run_bass_kernel_spmd(_NC, in_maps, core_ids=list(range(N_CORES)))
    total = sum(float(r["partial"].sum()) for r in res.results)
    return np.float32(-total / B)


if __name__ == "__main__":
    nc = build_bass()
    print("built OK:", len(nc.m.functions[0].allocations), "allocations")
